# revision 12
# baseline (speedup 1.0000x reference)
"""Minibatch discrimination kernel for Trainium2, 8 NeuronCores (SPMD).

Reference computation:
    M = (x @ T.reshape(F, O*I)).reshape(B, O, I)
    dist[a,b,o] = sum_i |M[a,o,i] - M[b,o,i]|
    o_feat[a,o] = sum_{b != a} exp(-dist[a,b,o])
    out = concat([x, o_feat], axis=1)            # [B, F+O]

Sharding: each of the 8 cores owns 32 rows of the `a` axis and computes
them against the full batch (M is recomputed per-core; T replicated).

Per-core device algorithm (B=256, F=1024, O=128, I=16):
  SBUF layout "M3"[p, g, b] with p = u*16 + i (u = o%8... actually o = 8g+u),
  i.e. partition packs (o_sub=8 x i=16), free packs (g=16 o-groups x b=256).
  - M3 built with 128 matmuls: lhsT = T[(k f-block), cols 128g:128g+128],
    rhs = x^T block -> psum [128, 256] accumulated over 8 k-blocks.
  - per (a, g): one fused |M3 - M3[:, g, a]| instruction
      DVE: tensor_scalar(op0=subtract, op1=abs_max vs 0)   (per-partition scalar)
      ACT: activation(Abs, scale=-1, bias=M3[:, g, a])     (split tunable)
  - the i-reduction (16 -> 1 per o) is a 0/1 selection matmul on TensorE
    accumulating dist[o, b] in PSUM over the 16 g-groups.
  - exp + partner sum: one ACT instruction: Exp(scale=-1) with accum_out,
    then subtract 1.0 (removes the b==a self term, exp(0)=1).

The distances here are O(100..1500) so exp underflows to 0 for every
off-diagonal pair; bf16 data paths are far more than accurate enough.
"""

from contextlib import ExitStack

import ml_dtypes
import numpy as np

import concourse.bacc as bacc
import concourse.bass as bass
import concourse.tile as tile
from concourse import mybir
from concourse._compat import with_exitstack
from concourse.bass_utils import run_bass_kernel_spmd

B, F, O, I = 256, 1024, 128, 16
NCORES = 8
SH = B // NCORES            # 32 "a" rows per core
G = O // 8                  # 16 o-groups of 8
KT = F // 128               # 8 contraction tiles
BF16 = mybir.dt.bfloat16
F32 = mybir.dt.float32
NPBF16 = ml_dtypes.bfloat16

G_ACT = 3                   # o-groups whose relu-units run on ScalarE (rest: VectorE)


@with_exitstack
def _body(ctx: ExitStack, tc: "tile.TileContext", xT_ap, Tb_ap, sel_ap, nhi_ap, out_ap):
    """relu reformulation:  |u-v| = 2*max(u,v) - u - v, and
    relu(Mb - Ma) = max(Ma,Mb) - Ma.  Per (a, o-group g) we compute one
    A[p,b] = relu(M3[p,b] - Ma[p]) tile (ScalarE or VectorE, one fused op
    each).  The selection matmul sums the 16 i's per o:
        psum[o,b] = X[o,b] - Sa[o],  X = sum_i max(Ma,Mb)
    one extra fp32 matmul (-1/2 I) @ S adds -Sb[o]/2, and then
        dist = 2X - Sa - Sb  =>  exp(-dist) = Exp(scale=-2, bias=-Sa)(psum).
    All quantities are exact fp32 sums of bf16 values, so dist(a,a) == 0
    exactly and the final "-1" removes the self term bit-exactly.
    """
    nc = tc.nc
    const = ctx.enter_context(tc.tile_pool(name="const", bufs=1))
    work = ctx.enter_context(tc.tile_pool(name="work", bufs=8))
    psum_m = ctx.enter_context(tc.tile_pool(name="psum_m", bufs=2, space="PSUM"))
    psum_d = ctx.enter_context(tc.tile_pool(name="psum_d", bufs=6, space="PSUM"))

    # ---- load inputs ----
    xsb = const.tile([128, KT, B], BF16)
    nc.sync.dma_start(out=xsb, in_=xT_ap.rearrange("(k p) b -> p k b", p=128))
    sel = const.tile([128, G, 128], BF16)
    nc.sync.dma_start(out=sel, in_=sel_ap.rearrange("p (g m) -> p g m", g=G))
    nhi = const.tile([128, 128], F32)
    nc.sync.dma_start(out=nhi, in_=nhi_ap)
    Tsb = const.tile([128, KT, O * I], BF16)
    nc.sync.dma_start(out=Tsb, in_=Tb_ap.rearrange("(k p) c -> p k c", p=128))

    # ---- M3[p=(u,i), (g, b)] via matmuls ----
    M3 = const.tile([128, G, B], BF16)
    # fp32 copies of the shard's own columns (per-partition scalars must be fp32),
    # taken from the bf16-rounded M3 so the self distance is exactly 0.
    Maf = const.tile([128, G, SH], F32)
    Mafn = const.tile([128, G, SH], F32)
    for g in range(G):
        ps = psum_m.tile([128, B], F32)
        for k in range(KT):
            nc.tensor.matmul(
                ps,
                lhsT=Tsb[:, k, bass.ts(g, 128)],
                rhs=xsb[:, k, :],
                start=(k == 0),
                stop=(k == KT - 1),
            )
        nc.scalar.copy(out=M3[:, g, :], in_=ps)
        nc.vector.tensor_copy(Maf[:, g, :], M3[:, g, :SH])
    nc.vector.tensor_scalar_mul(Mafn, Maf, -1.0)

    # ---- column sums S[o, b] = sum_i M[b, o, i] via the same selection MM ----
    sps = psum_m.tile([128, B], F32, tag="ps")
    for g in range(G):
        nc.tensor.matmul(
            sps, lhsT=sel[:, g, :], rhs=M3[:, g, :], start=(g == 0), stop=(g == G - 1)
        )
    Sf = const.tile([128, B], F32)
    nc.vector.tensor_copy(Sf, sps)
    nSaf = const.tile([128, SH], F32)
    nc.vector.tensor_scalar_mul(nSaf, Sf[:, :SH], -1.0)
    # duplicated [Sf | Sf] so one fp32 matmul corrects a pair of a-columns
    SfSf = const.tile([128, 2, B], F32)
    nc.vector.tensor_copy(SfSf[:, 0, :], Sf)
    nc.vector.tensor_copy(SfSf[:, 1, :], Sf)

    # ---- pairwise: relu units + selection matmuls + exp/accumulate ----
    # Waves of 8 'a' rows; two a's share one [128, 512] A-tile / psum bank so
    # each selection matmul covers two rows (N=512), and the stationary sel_g
    # is loaded once per (wave, g) instead of per matmul.
    ofeat = const.tile([128, SH], F32)
    WAVE = 8
    NP = WAVE // 2
    for w in range(SH // WAVE):
        pds = [
            psum_d.tile([128, 2 * B], F32, tag="pd", name=f"pd{w}_{i}")
            for i in range(NP)
        ]
        for g in range(G):
            for pi in range(NP):
                Ap = work.tile([128, 2 * B], BF16, tag="apair")
                for h in range(2):
                    a = w * WAVE + pi * 2 + h
                    dst = Ap[:, bass.ts(h, B)]
                    if g < G_ACT:
                        nc.scalar.activation(
                            dst, M3[:, g, :], mybir.ActivationFunctionType.Relu,
                            bias=Mafn[:, g, a : a + 1], scale=1.0,
                        )
                    else:
                        nc.vector.tensor_scalar(
                            dst, M3[:, g, :], Maf[:, g, a : a + 1], 0.0,
                            mybir.AluOpType.subtract, mybir.AluOpType.max,
                        )
                nc.tensor.matmul(
                    pds[pi], lhsT=sel[:, g, :], rhs=Ap, start=(g == 0), stop=False
                )
        for pi in range(NP):
            # psum += (-1/2 I) @ [S | S]   (fp32, exact)
            nc.tensor.matmul(pds[pi], lhsT=nhi, rhs=SfSf, start=False, stop=True)
        for pi in range(NP):
            for h in range(2):
                a = w * WAVE + pi * 2 + h
                sim = work.tile([128, B], BF16, tag="sim")
                nc.scalar.activation(
                    sim, pds[pi][:, bass.ts(h, B)],
                    mybir.ActivationFunctionType.Exp,
                    scale=-2.0, bias=nSaf[:, a : a + 1],
                    accum_out=ofeat[:, a : a + 1],
                )

    # remove self-similarity exp(0)=1 and write out
    ofn = const.tile([128, SH], F32)
    nc.vector.tensor_scalar_add(ofn, ofeat, -1.0)
    nc.sync.dma_start(out=out_ap, in_=ofn)


def _build_sel() -> np.ndarray:
    """sel[p, g*128 + m] = 1 iff m == 8*g + p//16  (sums i per o-group)."""
    sel = np.zeros((128, G, 128), dtype=np.float32)
    p = np.arange(128)
    for g in range(G):
        sel[p, g, 8 * g + p // 16] = 1.0
    return np.ascontiguousarray(sel.reshape(128, G * 128)).astype(NPBF16)


_CACHE: dict = {}


def _get_nc():
    if "nc" in _CACHE:
        return _CACHE["nc"]
    nc = bacc.Bacc("TRN2", target_bir_lowering=False, debug=False)
    xT = nc.dram_tensor("xT", [F, B], BF16, kind="ExternalInput")
    Tb = nc.dram_tensor("Tb", [F, O * I], BF16, kind="ExternalInput")
    sel = nc.dram_tensor("sel", [128, G * 128], BF16, kind="ExternalInput")
    nhi = nc.dram_tensor("nhi", [128, 128], F32, kind="ExternalInput")
    out = nc.dram_tensor("ofeatT", [128, SH], F32, kind="ExternalOutput")
    with tile.TileContext(nc) as tc:
        _body(tc, xT.ap(), Tb.ap(), sel.ap(), nhi.ap(), out.ap())
    nc.compile()
    _CACHE["nc"] = nc
    return nc


def _in_maps(x32: np.ndarray, T32: np.ndarray) -> list[dict]:
    Tb = np.ascontiguousarray(T32.reshape(F, O * I)).astype(NPBF16)
    sel = _build_sel()
    nhi = np.ascontiguousarray(-0.5 * np.eye(128, dtype=np.float32))
    maps = []
    for c in range(NCORES):
        xr = np.roll(x32, -SH * c, axis=0)  # this core's rows first
        maps.append(
            {
                "xT": np.ascontiguousarray(xr.T).astype(NPBF16),
                "Tb": Tb,
                "sel": sel,
                "nhi": nhi,
            }
        )
    return maps


def kernel(x: np.ndarray, T: np.ndarray, _bench_results=None) -> np.ndarray:
    x32 = np.ascontiguousarray(np.asarray(x), dtype=np.float32)
    T32 = np.ascontiguousarray(np.asarray(T), dtype=np.float32)
    nc = _get_nc()
    res = run_bass_kernel_spmd(nc, _in_maps(x32, T32), core_ids=list(range(NCORES)))
    if _bench_results is not None:
        _bench_results.append(res)
    ofeat = np.concatenate(
        [np.asarray(r["ofeatT"], np.float32).T for r in res.results], axis=0
    )  # [B, O]
    return np.concatenate([x32, ofeat], axis=1)


# revision 14
# speedup vs baseline: 1.3001x; 1.3001x over previous
"""Minibatch discrimination kernel for Trainium2, 8 NeuronCores (SPMD).

Reference computation:
    M = (x @ T.reshape(F, O*I)).reshape(B, O, I)
    dist[a,b,o] = sum_i |M[a,o,i] - M[b,o,i]|
    o_feat[a,o] = sum_{b != a} exp(-dist[a,b,o])
    out = concat([x, o_feat], axis=1)            # [B, F+O]

Sharding: each of the 8 cores owns 32 rows of the `a` axis and computes
them against the full batch (M is recomputed per-core; T replicated).

Per-core device algorithm (B=256, F=1024, O=128, I=16):
  SBUF layout "M3"[p, g, b] with p = u*16 + i (u = o%8... actually o = 8g+u),
  i.e. partition packs (o_sub=8 x i=16), free packs (g=16 o-groups x b=256).
  - M3 built with 128 matmuls: lhsT = T[(k f-block), cols 128g:128g+128],
    rhs = x^T block -> psum [128, 256] accumulated over 8 k-blocks.
  - per (a, g): one fused |M3 - M3[:, g, a]| instruction
      DVE: tensor_scalar(op0=subtract, op1=abs_max vs 0)   (per-partition scalar)
      ACT: activation(Abs, scale=-1, bias=M3[:, g, a])     (split tunable)
  - the i-reduction (16 -> 1 per o) is a 0/1 selection matmul on TensorE
    accumulating dist[o, b] in PSUM over the 16 g-groups.
  - exp + partner sum: one ACT instruction: Exp(scale=-1) with accum_out,
    then subtract 1.0 (removes the b==a self term, exp(0)=1).

The distances here are O(100..1500) so exp underflows to 0 for every
off-diagonal pair; bf16 data paths are far more than accurate enough.
"""

from contextlib import ExitStack

import ml_dtypes
import numpy as np

import concourse.bacc as bacc
import concourse.bass as bass
import concourse.tile as tile
from concourse import mybir
from concourse._compat import with_exitstack
from concourse.bass_utils import run_bass_kernel_spmd

B, F, O, I = 256, 1024, 128, 16
NCORES = 8
SH = B // NCORES            # 32 "a" rows per core
G = O // 8                  # 16 o-groups of 8
KT = F // 128               # 8 contraction tiles
BF16 = mybir.dt.bfloat16
F32 = mybir.dt.float32
NPBF16 = ml_dtypes.bfloat16

ACT_UNITS = 117             # of the 512 relu-units, how many run on ScalarE


@with_exitstack
def _body(ctx: ExitStack, tc: "tile.TileContext", xT_ap, Tb_ap, sel_ap, nhi_ap, out_ap):
    """relu reformulation:  |u-v| = 2*max(u,v) - u - v, and
    relu(Mb - Ma) = max(Ma,Mb) - Ma.  Per (a, o-group g) we compute one
    A[p,b] = relu(M3[p,b] - Ma[p]) tile (ScalarE or VectorE, one fused op
    each).  The selection matmul sums the 16 i's per o:
        psum[o,b] = X[o,b] - Sa[o],  X = sum_i max(Ma,Mb)
    one extra fp32 matmul (-1/2 I) @ S adds -Sb[o]/2, and then
        dist = 2X - Sa - Sb  =>  exp(-dist) = Exp(scale=-2, bias=-Sa)(psum).
    All quantities are exact fp32 sums of bf16 values, so dist(a,a) == 0
    exactly and the final "-1" removes the self term bit-exactly.
    """
    nc = tc.nc
    const = ctx.enter_context(tc.tile_pool(name="const", bufs=1))
    work = ctx.enter_context(tc.tile_pool(name="work", bufs=8))
    psum = ctx.enter_context(tc.tile_pool(name="psum", bufs=8, space="PSUM"))

    # ---- load inputs ----
    xsb = const.tile([128, KT, B], BF16)
    nc.sync.dma_start(out=xsb, in_=xT_ap.rearrange("(k p) b -> p k b", p=128))
    sel = const.tile([128, G, 128], BF16)
    nc.sync.dma_start(out=sel, in_=sel_ap.rearrange("p (g m) -> p g m", g=G))
    nhi = const.tile([128, 128], F32)
    nc.sync.dma_start(out=nhi, in_=nhi_ap)
    Tsb = const.tile([128, KT, O * I], BF16)
    nc.sync.dma_start(out=Tsb, in_=Tb_ap.rearrange("(k p) c -> p k c", p=128))

    # ---- M3[p=(u,i), (g, b)] via matmuls ----
    M3 = const.tile([128, G, B], BF16)
    # fp32 copies of the shard's own columns (per-partition scalars must be fp32),
    # taken from the bf16-rounded M3 so the self distance is exactly 0.
    Maf = const.tile([128, G, SH], F32)
    Mafn = const.tile([128, G, SH], F32)
    for g in range(G):
        ps = psum.tile([128, 2 * B], F32, tag="pd", name=f"mm{g}")
        for k in range(KT):
            nc.tensor.matmul(
                ps[:, :B],
                lhsT=Tsb[:, k, bass.ts(g, 128)],
                rhs=xsb[:, k, :],
                start=(k == 0),
                stop=(k == KT - 1),
            )
        nc.scalar.copy(out=M3[:, g, :], in_=ps[:, :B])
        nc.vector.tensor_copy(Maf[:, g, :], M3[:, g, :SH])
    nc.vector.tensor_scalar_mul(Mafn, Maf, -1.0)

    # ---- column sums S[o, b] = sum_i M[b, o, i] via the same selection MM ----
    sps = psum.tile([128, 2 * B], F32, tag="pd", name="sps")
    for g in range(G):
        nc.tensor.matmul(
            sps[:, :B], lhsT=sel[:, g, :], rhs=M3[:, g, :],
            start=(g == 0), stop=(g == G - 1)
        )
    Sf = const.tile([128, B], F32)
    nc.vector.tensor_copy(Sf, sps[:, :B])
    nSaf = const.tile([128, SH], F32)
    nc.vector.tensor_scalar_mul(nSaf, Sf[:, :SH], -1.0)
    # duplicated [Sf | Sf] so one fp32 matmul corrects a pair of a-columns
    SfSf = const.tile([128, 2, B], F32)
    nc.vector.tensor_copy(SfSf[:, 0, :], Sf)
    nc.vector.tensor_copy(SfSf[:, 1, :], Sf)

    # ---- pairwise: relu units + selection matmuls + exp/accumulate ----
    # Waves of 8 'a' rows; two a's share one [128, 512] A-tile / psum bank so
    # each selection matmul covers two rows (N=512), and the stationary sel_g
    # is loaded once per (wave, g) instead of per matmul.
    ofeat = const.tile([128, SH], F32)
    WAVE = 8
    NP = WAVE // 2
    for w in range(SH // WAVE):
        pds = [
            psum.tile([128, 2 * B], F32, tag="pd", name=f"pd{w}_{i}")
            for i in range(NP)
        ]
        for g in range(G):
            for pi in range(NP):
                Ap = work.tile([128, 2 * B], BF16, tag="apair")
                for h in range(2):
                    a = w * WAVE + pi * 2 + h
                    dst = Ap[:, bass.ts(h, B)]
                    unit = ((w * G + g) * NP + pi) * 2 + h
                    if (unit * ACT_UNITS) % 512 < ACT_UNITS:
                        nc.scalar.activation(
                            dst, M3[:, g, :], mybir.ActivationFunctionType.Relu,
                            bias=Mafn[:, g, a : a + 1], scale=1.0,
                        )
                    else:
                        nc.vector.tensor_scalar(
                            dst, M3[:, g, :], Maf[:, g, a : a + 1], 0.0,
                            mybir.AluOpType.subtract, mybir.AluOpType.max,
                        )
                nc.tensor.matmul(
                    pds[pi], lhsT=sel[:, g, :], rhs=Ap, start=(g == 0), stop=False
                )
        for pi in range(NP):
            # psum += (-1/2 I) @ [S | S]   (fp32, exact)
            nc.tensor.matmul(pds[pi], lhsT=nhi, rhs=SfSf, start=False, stop=True)
        for pi in range(NP):
            for h in range(2):
                a = w * WAVE + pi * 2 + h
                sim = work.tile([128, B], BF16, tag="sim")
                nc.scalar.activation(
                    sim, pds[pi][:, bass.ts(h, B)],
                    mybir.ActivationFunctionType.Exp,
                    scale=-2.0, bias=nSaf[:, a : a + 1],
                    accum_out=ofeat[:, a : a + 1],
                )

    # remove self-similarity exp(0)=1 and write out
    ofn = const.tile([128, SH], F32)
    nc.vector.tensor_scalar_add(ofn, ofeat, -1.0)
    nc.sync.dma_start(out=out_ap, in_=ofn)


def _build_sel() -> np.ndarray:
    """sel[p, g*128 + m] = 1 iff m == 8*g + p//16  (sums i per o-group)."""
    sel = np.zeros((128, G, 128), dtype=np.float32)
    p = np.arange(128)
    for g in range(G):
        sel[p, g, 8 * g + p // 16] = 1.0
    return np.ascontiguousarray(sel.reshape(128, G * 128)).astype(NPBF16)


_CACHE: dict = {}


def _get_nc():
    if "nc" in _CACHE:
        return _CACHE["nc"]
    nc = bacc.Bacc("TRN2", target_bir_lowering=False, debug=False)
    xT = nc.dram_tensor("xT", [F, B], BF16, kind="ExternalInput")
    Tb = nc.dram_tensor("Tb", [F, O * I], BF16, kind="ExternalInput")
    sel = nc.dram_tensor("sel", [128, G * 128], BF16, kind="ExternalInput")
    nhi = nc.dram_tensor("nhi", [128, 128], F32, kind="ExternalInput")
    out = nc.dram_tensor("ofeatT", [128, SH], F32, kind="ExternalOutput")
    with tile.TileContext(nc) as tc:
        _body(tc, xT.ap(), Tb.ap(), sel.ap(), nhi.ap(), out.ap())
    nc.compile()
    _CACHE["nc"] = nc
    return nc


def _in_maps(x32: np.ndarray, T32: np.ndarray) -> list[dict]:
    Tb = np.ascontiguousarray(T32.reshape(F, O * I)).astype(NPBF16)
    sel = _build_sel()
    nhi = np.ascontiguousarray(-0.5 * np.eye(128, dtype=np.float32))
    maps = []
    for c in range(NCORES):
        xr = np.roll(x32, -SH * c, axis=0)  # this core's rows first
        maps.append(
            {
                "xT": np.ascontiguousarray(xr.T).astype(NPBF16),
                "Tb": Tb,
                "sel": sel,
                "nhi": nhi,
            }
        )
    return maps


def kernel(x: np.ndarray, T: np.ndarray, _bench_results=None) -> np.ndarray:
    x32 = np.ascontiguousarray(np.asarray(x), dtype=np.float32)
    T32 = np.ascontiguousarray(np.asarray(T), dtype=np.float32)
    nc = _get_nc()
    res = run_bass_kernel_spmd(nc, _in_maps(x32, T32), core_ids=list(range(NCORES)))
    if _bench_results is not None:
        _bench_results.append(res)
    ofeat = np.concatenate(
        [np.asarray(r["ofeatT"], np.float32).T for r in res.results], axis=0
    )  # [B, O]
    return np.concatenate([x32, ofeat], axis=1)


# revision 16
# speedup vs baseline: 1.3519x; 1.0398x over previous
"""Minibatch discrimination kernel for Trainium2, 8 NeuronCores (SPMD).

Reference computation:
    M = (x @ T.reshape(F, O*I)).reshape(B, O, I)
    dist[a,b,o] = sum_i |M[a,o,i] - M[b,o,i]|
    o_feat[a,o] = sum_{b != a} exp(-dist[a,b,o])
    out = concat([x, o_feat], axis=1)            # [B, F+O]

Sharding: each of the 8 cores owns 32 rows of the `a` axis and computes
them against the full batch (M is recomputed per-core; T replicated).

Per-core device algorithm (B=256, F=1024, O=128, I=16):
  SBUF layout "M3"[p, g, b] with p = u*16 + i (u = o%8... actually o = 8g+u),
  i.e. partition packs (o_sub=8 x i=16), free packs (g=16 o-groups x b=256).
  - M3 built with 128 matmuls: lhsT = T[(k f-block), cols 128g:128g+128],
    rhs = x^T block -> psum [128, 256] accumulated over 8 k-blocks.
  - per (a, g): one fused |M3 - M3[:, g, a]| instruction
      DVE: tensor_scalar(op0=subtract, op1=abs_max vs 0)   (per-partition scalar)
      ACT: activation(Abs, scale=-1, bias=M3[:, g, a])     (split tunable)
  - the i-reduction (16 -> 1 per o) is a 0/1 selection matmul on TensorE
    accumulating dist[o, b] in PSUM over the 16 g-groups.
  - exp + partner sum: one ACT instruction: Exp(scale=-1) with accum_out,
    then subtract 1.0 (removes the b==a self term, exp(0)=1).

The distances here are O(100..1500) so exp underflows to 0 for every
off-diagonal pair; bf16 data paths are far more than accurate enough.
"""

from contextlib import ExitStack

import ml_dtypes
import numpy as np

import concourse.bacc as bacc
import concourse.bass as bass
import concourse.tile as tile
from concourse import mybir
from concourse._compat import with_exitstack
from concourse.bass_utils import run_bass_kernel_spmd

B, F, O, I = 256, 1024, 128, 16
NCORES = 8
SH = B // NCORES            # 32 "a" rows per core
G = O // 8                  # 16 o-groups of 8
KT = F // 128               # 8 contraction tiles
BF16 = mybir.dt.bfloat16
F32 = mybir.dt.float32
NPBF16 = ml_dtypes.bfloat16

ACT_UNITS = 117             # of the 512 relu-units, how many run on ScalarE


@with_exitstack
def _body(ctx: ExitStack, tc: "tile.TileContext", xT_ap, Tb_ap, sel_ap, nhi_ap, out_ap):
    """relu reformulation:  |u-v| = 2*max(u,v) - u - v, and
    relu(Mb - Ma) = max(Ma,Mb) - Ma.  Per (a, o-group g) we compute one
    A[p,b] = relu(M3[p,b] - Ma[p]) tile (ScalarE or VectorE, one fused op
    each).  The selection matmul sums the 16 i's per o:
        psum[o,b] = X[o,b] - Sa[o],  X = sum_i max(Ma,Mb)
    one extra fp32 matmul (-1/2 I) @ S adds -Sb[o]/2, and then
        dist = 2X - Sa - Sb  =>  exp(-dist) = Exp(scale=-2, bias=-Sa)(psum).
    All quantities are exact fp32 sums of bf16 values, so dist(a,a) == 0
    exactly and the final "-1" removes the self term bit-exactly.
    """
    nc = tc.nc
    const = ctx.enter_context(tc.tile_pool(name="const", bufs=1))
    work = ctx.enter_context(tc.tile_pool(name="work", bufs=8))
    psum = ctx.enter_context(tc.tile_pool(name="psum", bufs=8, space="PSUM"))

    # ---- load inputs ----
    xsb = const.tile([128, KT, B], BF16)
    nc.sync.dma_start(out=xsb, in_=xT_ap.rearrange("(k p) b -> p k b", p=128))
    sel = const.tile([128, G, 128], BF16)
    nc.sync.dma_start(out=sel, in_=sel_ap.rearrange("p (g m) -> p g m", g=G))
    nhi = const.tile([128, 128], F32)
    nc.sync.dma_start(out=nhi, in_=nhi_ap)
    Tsb = const.tile([128, KT, O * I], BF16)
    Tb_k = Tb_ap.rearrange("(k p) c -> k p c", p=128)
    for k in range(KT):
        nc.sync.dma_start(out=Tsb[:, k, :], in_=Tb_k[k])

    # ---- M3[p=(u,i), (g, b)] via matmuls ----
    M3 = const.tile([128, G, B], BF16)
    # fp32 copies of the shard's own columns (per-partition scalars must be fp32),
    # taken from the bf16-rounded M3 so the self distance is exactly 0.
    Maf = const.tile([128, G, SH], F32)
    Mafn = const.tile([128, G, SH], F32)
    for g in range(G):
        ps = psum.tile([128, 2 * B], F32, tag="pd", name=f"mm{g}")
        for k in range(KT):
            nc.tensor.matmul(
                ps[:, :B],
                lhsT=Tsb[:, k, bass.ts(g, 128)],
                rhs=xsb[:, k, :],
                start=(k == 0),
                stop=(k == KT - 1),
            )
        nc.scalar.copy(out=M3[:, g, :], in_=ps[:, :B])
        nc.vector.tensor_copy(Maf[:, g, :], M3[:, g, :SH])
    nc.vector.tensor_scalar_mul(Mafn, Maf, -1.0)

    # ---- column sums S[o, b] = sum_i M[b, o, i] via the same selection MM ----
    sps = psum.tile([128, 2 * B], F32, tag="pd", name="sps")
    for g in range(G):
        nc.tensor.matmul(
            sps[:, :B], lhsT=sel[:, g, :], rhs=M3[:, g, :],
            start=(g == 0), stop=(g == G - 1)
        )
    Sf = const.tile([128, B], F32)
    nc.vector.tensor_copy(Sf, sps[:, :B])
    nSaf = const.tile([128, SH], F32)
    nc.vector.tensor_scalar_mul(nSaf, Sf[:, :SH], -1.0)
    # duplicated [Sf | Sf] so one fp32 matmul corrects a pair of a-columns
    SfSf = const.tile([128, 2, B], F32)
    nc.vector.tensor_copy(SfSf[:, 0, :], Sf)
    nc.vector.tensor_copy(SfSf[:, 1, :], Sf)

    # ---- pairwise: relu units + selection matmuls + exp/accumulate ----
    # Waves of 8 'a' rows; two a's share one [128, 512] A-tile / psum bank so
    # each selection matmul covers two rows (N=512), and the stationary sel_g
    # is loaded once per (wave, g) instead of per matmul.
    ofeat = const.tile([128, SH], F32)
    WAVE = 8
    NP = WAVE // 2
    for w in range(SH // WAVE):
        pds = [
            psum.tile([128, 2 * B], F32, tag="pd", name=f"pd{w}_{i}")
            for i in range(NP)
        ]
        # Four o-strips live in different 32-column groups of the PE array, so
        # the four selection matmuls of round r run concurrently (col tiling).
        for r in range(4):
            for pi in range(NP):
                for jj in range(4):
                    g = 4 * jj + r
                    Ap = work.tile([128, 2 * B], BF16, tag="apair", name=f"ap{w}_{r}_{pi}_{jj}")
                    for h in range(2):
                        a = w * WAVE + pi * 2 + h
                        dst = Ap[:, bass.ts(h, B)]
                        unit = ((w * G + g) * NP + pi) * 2 + h
                        if (unit * ACT_UNITS) % 512 < ACT_UNITS:
                            nc.scalar.activation(
                                dst, M3[:, g, :], mybir.ActivationFunctionType.Relu,
                                bias=Mafn[:, g, a : a + 1], scale=1.0,
                            )
                        else:
                            nc.vector.tensor_scalar(
                                dst, M3[:, g, :], Maf[:, g, a : a + 1], 0.0,
                                mybir.AluOpType.subtract, mybir.AluOpType.max,
                            )
                    nc.tensor.matmul(
                        pds[pi][bass.ts(jj, 32), :],
                        lhsT=sel[:, g, bass.ts(jj, 32)],
                        rhs=Ap,
                        start=(r == 0),
                        stop=(r == 3),
                        tile_position=(0, 32 * jj),
                    )
        for pi in range(NP):
            # psum += (-1/2 I) @ [S | S]   (fp32, exact)
            nc.tensor.matmul(
                pds[pi], lhsT=nhi, rhs=SfSf, start=False, stop=True,
                skip_group_check=True,
            )
        for pi in range(NP):
            for h in range(2):
                a = w * WAVE + pi * 2 + h
                sim = work.tile([128, B], BF16, tag="sim")
                nc.scalar.activation(
                    sim, pds[pi][:, bass.ts(h, B)],
                    mybir.ActivationFunctionType.Exp,
                    scale=-2.0, bias=nSaf[:, a : a + 1],
                    accum_out=ofeat[:, a : a + 1],
                )

    # remove self-similarity exp(0)=1 and write out
    ofn = const.tile([128, SH], F32)
    nc.vector.tensor_scalar_add(ofn, ofeat, -1.0)
    nc.sync.dma_start(out=out_ap, in_=ofn)


def _build_sel() -> np.ndarray:
    """sel[p, g*128 + m] = 1 iff m == 8*g + p//16  (sums i per o-group)."""
    sel = np.zeros((128, G, 128), dtype=np.float32)
    p = np.arange(128)
    for g in range(G):
        sel[p, g, 8 * g + p // 16] = 1.0
    return np.ascontiguousarray(sel.reshape(128, G * 128)).astype(NPBF16)


_CACHE: dict = {}


def _get_nc():
    if "nc" in _CACHE:
        return _CACHE["nc"]
    nc = bacc.Bacc("TRN2", target_bir_lowering=False, debug=False)
    xT = nc.dram_tensor("xT", [F, B], BF16, kind="ExternalInput")
    Tb = nc.dram_tensor("Tb", [F, O * I], BF16, kind="ExternalInput")
    sel = nc.dram_tensor("sel", [128, G * 128], BF16, kind="ExternalInput")
    nhi = nc.dram_tensor("nhi", [128, 128], F32, kind="ExternalInput")
    out = nc.dram_tensor("ofeatT", [128, SH], F32, kind="ExternalOutput")
    with tile.TileContext(nc) as tc:
        _body(tc, xT.ap(), Tb.ap(), sel.ap(), nhi.ap(), out.ap())
    nc.compile()
    _CACHE["nc"] = nc
    return nc


def _in_maps(x32: np.ndarray, T32: np.ndarray) -> list[dict]:
    Tb = np.ascontiguousarray(T32.reshape(F, O * I)).astype(NPBF16)
    sel = _build_sel()
    nhi = np.ascontiguousarray(-0.5 * np.eye(128, dtype=np.float32))
    maps = []
    for c in range(NCORES):
        xr = np.roll(x32, -SH * c, axis=0)  # this core's rows first
        maps.append(
            {
                "xT": np.ascontiguousarray(xr.T).astype(NPBF16),
                "Tb": Tb,
                "sel": sel,
                "nhi": nhi,
            }
        )
    return maps


def kernel(x: np.ndarray, T: np.ndarray, _bench_results=None) -> np.ndarray:
    x32 = np.ascontiguousarray(np.asarray(x), dtype=np.float32)
    T32 = np.ascontiguousarray(np.asarray(T), dtype=np.float32)
    nc = _get_nc()
    res = run_bass_kernel_spmd(nc, _in_maps(x32, T32), core_ids=list(range(NCORES)))
    if _bench_results is not None:
        _bench_results.append(res)
    ofeat = np.concatenate(
        [np.asarray(r["ofeatT"], np.float32).T for r in res.results], axis=0
    )  # [B, O]
    return np.concatenate([x32, ofeat], axis=1)


# revision 18
# speedup vs baseline: 1.3836x; 1.0235x over previous
"""Minibatch discrimination kernel for Trainium2, 8 NeuronCores (SPMD).

Reference computation:
    M = (x @ T.reshape(F, O*I)).reshape(B, O, I)
    dist[a,b,o] = sum_i |M[a,o,i] - M[b,o,i]|
    o_feat[a,o] = sum_{b != a} exp(-dist[a,b,o])
    out = concat([x, o_feat], axis=1)            # [B, F+O]

Sharding: each of the 8 cores owns 32 rows of the `a` axis and computes
them against the full batch (M is recomputed per-core; T replicated).

Per-core device algorithm (B=256, F=1024, O=128, I=16):
  SBUF layout "M3"[p, g, b] with p = u*16 + i (u = o%8... actually o = 8g+u),
  i.e. partition packs (o_sub=8 x i=16), free packs (g=16 o-groups x b=256).
  - M3 built with 128 matmuls: lhsT = T[(k f-block), cols 128g:128g+128],
    rhs = x^T block -> psum [128, 256] accumulated over 8 k-blocks.
  - per (a, g): one fused |M3 - M3[:, g, a]| instruction
      DVE: tensor_scalar(op0=subtract, op1=abs_max vs 0)   (per-partition scalar)
      ACT: activation(Abs, scale=-1, bias=M3[:, g, a])     (split tunable)
  - the i-reduction (16 -> 1 per o) is a 0/1 selection matmul on TensorE
    accumulating dist[o, b] in PSUM over the 16 g-groups.
  - exp + partner sum: one ACT instruction: Exp(scale=-1) with accum_out,
    then subtract 1.0 (removes the b==a self term, exp(0)=1).

The distances here are O(100..1500) so exp underflows to 0 for every
off-diagonal pair; bf16 data paths are far more than accurate enough.
"""

from contextlib import ExitStack

import ml_dtypes
import numpy as np

import concourse.bacc as bacc
import concourse.bass as bass
import concourse.tile as tile
from concourse import mybir
from concourse._compat import with_exitstack
from concourse.bass_utils import run_bass_kernel_spmd

B, F, O, I = 256, 1024, 128, 16
NCORES = 8
SH = B // NCORES            # 32 "a" rows per core
G = O // 8                  # 16 o-groups of 8
KT = F // 128               # 8 contraction tiles
BF16 = mybir.dt.bfloat16
F32 = mybir.dt.float32
NPBF16 = ml_dtypes.bfloat16

ACT_UNITS = 117             # of the 512 relu-units, how many run on ScalarE


@with_exitstack
def _body(ctx: ExitStack, tc: "tile.TileContext", xT_ap, Tb_ap, sel_ap, nhi_ap, out_ap):
    """relu reformulation:  |u-v| = 2*max(u,v) - u - v, and
    relu(Mb - Ma) = max(Ma,Mb) - Ma.  Per (a, o-group g) we compute one
    A[p,b] = relu(M3[p,b] - Ma[p]) tile (ScalarE or VectorE, one fused op
    each).  The selection matmul sums the 16 i's per o:
        psum[o,b] = X[o,b] - Sa[o],  X = sum_i max(Ma,Mb)
    one extra fp32 matmul (-1/2 I) @ S adds -Sb[o]/2, and then
        dist = 2X - Sa - Sb  =>  exp(-dist) = Exp(scale=-2, bias=-Sa)(psum).
    All quantities are exact fp32 sums of bf16 values, so dist(a,a) == 0
    exactly and the final "-1" removes the self term bit-exactly.
    """
    nc = tc.nc
    const = ctx.enter_context(tc.tile_pool(name="const", bufs=1))
    work = ctx.enter_context(tc.tile_pool(name="work", bufs=8))
    psum = ctx.enter_context(tc.tile_pool(name="psum", bufs=8, space="PSUM"))

    # ---- load inputs ----
    xsb = const.tile([128, KT, B], BF16)
    nc.sync.dma_start(out=xsb, in_=xT_ap.rearrange("(k p) b -> p k b", p=128))
    sel = const.tile([128, G, 128], BF16)
    nc.sync.dma_start(out=sel, in_=sel_ap.rearrange("p (g m) -> p g m", g=G))
    nhi = const.tile([128, 128], F32)
    nc.sync.dma_start(out=nhi, in_=nhi_ap)
    Tsb = const.tile([128, KT, O * I], BF16)
    Tb_k = Tb_ap.rearrange("(k p) c -> k p c", p=128)
    for k in range(KT):
        nc.sync.dma_start(out=Tsb[:, k, :], in_=Tb_k[k])

    # ---- M3[p=(u,i), (g, b)] via matmuls ----
    M3 = const.tile([128, G, B], BF16)
    # fp32 copies of the shard's own columns (per-partition scalars must be fp32),
    # taken from the bf16-rounded M3 so the self distance is exactly 0.
    Maf = const.tile([128, G, SH], F32)
    Mafn = const.tile([128, G, SH], F32)
    for g in range(G):
        ps = psum.tile([128, 2 * B], F32, tag="pd", name=f"mm{g}")
        for k in range(KT):
            nc.tensor.matmul(
                ps[:, :B],
                lhsT=Tsb[:, k, bass.ts(g, 128)],
                rhs=xsb[:, k, :],
                start=(k == 0),
                stop=(k == KT - 1),
            )
        nc.scalar.copy(out=M3[:, g, :], in_=ps[:, :B])
        nc.vector.tensor_copy(Maf[:, g, :], M3[:, g, :SH])
    nc.vector.tensor_scalar_mul(Mafn, Maf, -1.0)

    # ---- column sums S[o, b] = sum_i M[b, o, i] via the same selection MM ----
    sps = psum.tile([128, 2 * B], F32, tag="pd", name="sps")
    for g in range(G):
        nc.tensor.matmul(
            sps[:, :B], lhsT=sel[:, g, :], rhs=M3[:, g, :],
            start=(g == 0), stop=(g == G - 1)
        )
    Sf = const.tile([128, B], F32)
    nc.vector.tensor_copy(Sf, sps[:, :B])
    nSaf = const.tile([128, SH], F32)
    nc.vector.tensor_scalar_mul(nSaf, Sf[:, :SH], -1.0)
    # duplicated [Sf | Sf] so one fp32 matmul corrects a pair of a-columns
    SfSf = const.tile([128, 2, B], F32)
    nc.vector.tensor_copy(SfSf[:, 0, :], Sf)
    nc.vector.tensor_copy(SfSf[:, 1, :], Sf)

    # ---- pairwise: relu units + selection matmuls + exp/accumulate ----
    # Waves of 8 'a' rows; two a's share one [128, 512] A-tile / psum bank so
    # each selection matmul covers two rows (N=512), and the stationary sel_g
    # is loaded once per (wave, g) instead of per matmul.
    ofeat = const.tile([128, SH], F32)
    WAVE = 8
    NP = WAVE // 2
    for w in range(SH // WAVE):
        pds = [
            psum.tile([128, 2 * B], F32, tag="pd", name=f"pd{w}_{i}")
            for i in range(NP)
        ]
        # Zero + apply the -Sb/2 correction first: one full-width fp32 matmul
        # (start=True clears the bank), then all strip matmuls accumulate.
        for pi in range(NP):
            nc.tensor.matmul(pds[pi], lhsT=nhi, rhs=SfSf, start=True, stop=False)
        # Four o-strips live in different 32-column groups of the PE array, so
        # the four selection matmuls of round r run concurrently (col tiling).
        for r in range(4):
            for pi in range(NP):
                for jj in range(4):
                    g = 4 * jj + r
                    Ap = work.tile([128, 2 * B], BF16, tag="apair", name=f"ap{w}_{r}_{pi}_{jj}")
                    for h in range(2):
                        a = w * WAVE + pi * 2 + h
                        dst = Ap[:, bass.ts(h, B)]
                        unit = ((w * G + g) * NP + pi) * 2 + h
                        if (unit * ACT_UNITS) % 512 < ACT_UNITS:
                            nc.scalar.activation(
                                dst, M3[:, g, :], mybir.ActivationFunctionType.Relu,
                                bias=Mafn[:, g, a : a + 1], scale=1.0,
                            )
                        else:
                            nc.vector.tensor_scalar(
                                dst, M3[:, g, :], Maf[:, g, a : a + 1], 0.0,
                                mybir.AluOpType.subtract, mybir.AluOpType.max,
                            )
                    nc.tensor.matmul(
                        pds[pi][bass.ts(jj, 32), :],
                        lhsT=sel[:, g, bass.ts(jj, 32)],
                        rhs=Ap,
                        start=False,
                        stop=(r == 3),
                        tile_position=(0, 32 * jj),
                        skip_group_check=True,
                    )
        for pi in range(NP):
            for h in range(2):
                a = w * WAVE + pi * 2 + h
                sim = work.tile([128, B], BF16, tag="sim")
                nc.scalar.activation(
                    sim, pds[pi][:, bass.ts(h, B)],
                    mybir.ActivationFunctionType.Exp,
                    scale=-2.0, bias=nSaf[:, a : a + 1],
                    accum_out=ofeat[:, a : a + 1],
                )

    # remove self-similarity exp(0)=1 and write out
    ofn = const.tile([128, SH], F32)
    nc.vector.tensor_scalar_add(ofn, ofeat, -1.0)
    nc.sync.dma_start(out=out_ap, in_=ofn)


def _build_sel() -> np.ndarray:
    """sel[p, g*128 + m] = 1 iff m == 8*g + p//16  (sums i per o-group)."""
    sel = np.zeros((128, G, 128), dtype=np.float32)
    p = np.arange(128)
    for g in range(G):
        sel[p, g, 8 * g + p // 16] = 1.0
    return np.ascontiguousarray(sel.reshape(128, G * 128)).astype(NPBF16)


_CACHE: dict = {}


def _get_nc():
    if "nc" in _CACHE:
        return _CACHE["nc"]
    nc = bacc.Bacc("TRN2", target_bir_lowering=False, debug=False)
    xT = nc.dram_tensor("xT", [F, B], BF16, kind="ExternalInput")
    Tb = nc.dram_tensor("Tb", [F, O * I], BF16, kind="ExternalInput")
    sel = nc.dram_tensor("sel", [128, G * 128], BF16, kind="ExternalInput")
    nhi = nc.dram_tensor("nhi", [128, 128], F32, kind="ExternalInput")
    out = nc.dram_tensor("ofeatT", [128, SH], F32, kind="ExternalOutput")
    with tile.TileContext(nc) as tc:
        _body(tc, xT.ap(), Tb.ap(), sel.ap(), nhi.ap(), out.ap())
    nc.compile()
    _CACHE["nc"] = nc
    return nc


def _in_maps(x32: np.ndarray, T32: np.ndarray) -> list[dict]:
    Tb = np.ascontiguousarray(T32.reshape(F, O * I)).astype(NPBF16)
    sel = _build_sel()
    nhi = np.ascontiguousarray(-0.5 * np.eye(128, dtype=np.float32))
    maps = []
    for c in range(NCORES):
        xr = np.roll(x32, -SH * c, axis=0)  # this core's rows first
        maps.append(
            {
                "xT": np.ascontiguousarray(xr.T).astype(NPBF16),
                "Tb": Tb,
                "sel": sel,
                "nhi": nhi,
            }
        )
    return maps


def kernel(x: np.ndarray, T: np.ndarray, _bench_results=None) -> np.ndarray:
    x32 = np.ascontiguousarray(np.asarray(x), dtype=np.float32)
    T32 = np.ascontiguousarray(np.asarray(T), dtype=np.float32)
    nc = _get_nc()
    res = run_bass_kernel_spmd(nc, _in_maps(x32, T32), core_ids=list(range(NCORES)))
    if _bench_results is not None:
        _bench_results.append(res)
    ofeat = np.concatenate(
        [np.asarray(r["ofeatT"], np.float32).T for r in res.results], axis=0
    )  # [B, O]
    return np.concatenate([x32, ofeat], axis=1)
